# revision 17
# baseline (speedup 1.0000x reference)
"""Delayed synaptic layer on 8 Trainium2 NeuronCores.

Math: out[b,q] = sum_p weight[p,q] * interp(buf[b,:,p], d[p,q]),
      d = 50*sigmoid(delay_raw), interp = linear interpolation over t.

Key restructure (exact identity): with clip01(x) = min(max(x,0),1),
the tent interpolation kernel satisfies tent(d-t) = clip01(d-t+1) - clip01(d-t), so

  out = buf[:,0,:] @ W + sum_{s=0}^{49} (buf[:,s+1,:]-buf[:,s,:]) @ (W * clip01(d-s))

This replaces the per-synapse gather with 50 dense clamp+multiply passes and
accumulating matmuls -- no gathers, no one-hots. clip01 values live in [0,1]
(mostly exactly 0 or 1), so fp16 operand storage adds no error amplification
(a raw-relu basis would amplify fp16 rounding ~100x through cancellation).

Engine schedule per step (per core, steady state ~3.6us/step):
  ScalarE : u = relu(d - j)            (1 elem/lane/cyc, the bottleneck)
  VectorE : c = min(u, 1)              (4x single-src mode)
            r = c * w                  (2x fp16 tensor_tensor)
  TensorE : psum[strip] += gT_j.T @ r  (16 matmuls, packed 4-wide into the
            128-wide array via tile_position col strips; M=16 per strip)
The four col-strip partial sums are added at the end. ~213us/core measured,
vs ~30us memory roofline; ScalarE+VectorE run ~97% busy.

Sharding: columns (n_post) split across the 8 cores; buf replicated; host
does layout/dtype prep only (transpose + fp16 cast), all arithmetic on-device.
"""

import numpy as np

B, T, P, QFULL = 16, 51, 2048, 2048
NCORES = 8
Q = QFULL // NCORES          # 256 output columns per core
NPT = P // 128               # 16 partition tiles over pre-neurons
NS = T - 1                   # 50 clip terms
FD = NPT * Q                 # 4096 free-dim elements per [128, .] pass

_CACHE = {}

# dtype of the clamp/multiply chain (w, clip, rhs). Measured: the DVE STT
# uop is 1x for BOTH fp16 and bf16, so there is no speed reason for bf16;
# fp16 is ~3x more accurate. Keep False.
STT_BF16 = False


def _build_program():
    import concourse.bass as bass
    import concourse.mybir as mybir
    from concourse.tile import TileContext

    fp32 = mybir.dt.float32
    fp16 = mybir.dt.float16
    sttdt = mybir.dt.bfloat16 if STT_BF16 else fp16
    Act = mybir.ActivationFunctionType
    Alu = mybir.AluOpType

    nc = bass.Bass()
    buft_d = nc.dram_tensor("buft", [128, NPT * T * B], fp16, kind="ExternalInput")
    w_d = nc.dram_tensor("w", [128, FD], sttdt, kind="ExternalInput")
    delay_d = nc.dram_tensor("delay", [128, FD], fp32, kind="ExternalInput")
    out_d = nc.dram_tensor("out", [B, Q], fp32, kind="ExternalOutput")

    with TileContext(nc) as tc:
        with (
            tc.tile_pool(name="persist", bufs=1) as persist,
            tc.tile_pool(name="upool", bufs=3) as upool,
            tc.tile_pool(name="rpool", bufs=3) as rpool,
            tc.tile_pool(name="cpool", bufs=3) as cpool,
            tc.tile_pool(name="psump", bufs=1, space="PSUM") as psump,
        ):
            buft = persist.tile([128, NPT * T * B], fp16, tag="buft")
            w = persist.tile([128, FD], sttdt, tag="w")
            delay = persist.tile([128, FD], fp32, tag="delay")
            # delay first: sigmoid -> d50 -> first relu is the critical path
            # into the steady-state loop; buft/w are not needed until the
            # first matmul/multiply ~10us later. DMA + sigmoid + x50 are
            # chunked 4-way so the first relu starts as early as possible.
            sg = persist.tile([128, FD], fp32, tag="sg")
            d50 = persist.tile([128, FD], fp32, tag="d50")
            H = FD // 4
            for h in range(4):
                sl = slice(h * H, (h + 1) * H)
                nc.sync.dma_start(out=delay[:, sl], in_=delay_d[:, sl])
            nc.sync.dma_start(out=buft[:], in_=buft_d[:])
            nc.sync.dma_start(out=w[:], in_=w_d[:])
            for h in range(4):
                sl = slice(h * H, (h + 1) * H)
                nc.scalar.activation(sg[:, sl], delay[:, sl], Act.Sigmoid)
                nc.vector.tensor_scalar_mul(d50[:, sl], sg[:, sl], 50.0)

            # per-step activation bias column j holds -j (ACT bias must be an AP)
            bias_i = persist.tile([128, NS], mybir.dt.int32, tag="bias_i")
            nc.gpsimd.iota(bias_i[:], pattern=[[1, NS]], base=0, channel_multiplier=0)
            bias_f = persist.tile([128, NS], fp32, tag="bias_f")
            nc.vector.tensor_scalar_mul(bias_f[:], bias_i[:], -1.0)

            # gT[pr, pt, s, b] = buf[b, s+1, p] - buf[b, s, p]   (p = pt*128+pr)
            # Only the first few s are computed up front: the full subtract
            # (6.8us on DVE) would gate the loop's first min/mult; the bulk
            # is emitted after step 2 below, where the pipeline absorbs it.
            GSPLIT = 6
            buft_v = buft[:].rearrange("p (pt t b) -> p pt t b", pt=NPT, t=T, b=B)
            gT = persist.tile([128, NPT * NS * B], fp16, tag="gT")
            gT_v = gT[:].rearrange("p (pt s b) -> p pt s b", pt=NPT, s=NS, b=B)
            nc.vector.tensor_tensor(
                gT_v[:, :, :GSPLIT, :],
                buft_v[:, :, 1 : GSPLIT + 1, :],
                buft_v[:, :, :GSPLIT, :],
                Alu.subtract,
            )

            # absorb the w DMA-completion wait on a cheap DVE op so the first
            # multiply in the loop below doesn't have to carry it
            wtouch = persist.tile([128, 2], sttdt, tag="wtouch")
            nc.vector.tensor_copy(wtouch[:], w[:, 0:2])

            psum = psump.tile([128, Q], fp32, tag="acc")

            # constant term: buf[:,0,:] @ W; also opens each col-strip's
            # accumulation group
            for pt in range(NPT):
                strip = pt % 4
                nc.tensor.matmul(
                    psum[32 * strip : 32 * strip + B, :],
                    lhsT=buft_v[:, pt, 0, :],
                    rhs=w[:, pt * Q : (pt + 1) * Q],
                    start=(pt < 4),
                    stop=False,
                    tile_position=(0, 32 * strip),
                    skip_group_check=True,
                )

            for j in range(NS):
                # u = relu(d50 - j) on ScalarE (1x), clamp top on DVE at 4x,
                # multiply on DVE at 2x. STT would fuse clamp+mult but its
                # uop only runs 1x, which is slower overall.
                u = upool.tile([128, FD], sttdt, tag="u")
                nc.scalar.activation(u[:], d50[:], Act.Relu, bias=bias_f[:, j : j + 1])
                c = cpool.tile([128, FD], sttdt, tag="c")
                nc.vector.tensor_scalar(c[:], u[:], 1.0, None, Alu.min)
                r = rpool.tile([128, FD], sttdt, tag="rhs")
                nc.vector.tensor_tensor(r[:], c[:], w[:], Alu.mult)
                if j == 2:
                    nc.vector.tensor_tensor(
                        gT_v[:, :, GSPLIT:, :],
                        buft_v[:, :, GSPLIT + 1 :, :],
                        buft_v[:, :, GSPLIT:NS, :],
                        Alu.subtract,
                    )
                last = j == NS - 1
                for pt in range(NPT):
                    strip = pt % 4
                    nc.tensor.matmul(
                        psum[32 * strip : 32 * strip + B, :],
                        lhsT=gT_v[:, pt, j, :],
                        rhs=r[:, pt * Q : (pt + 1) * Q],
                        start=False,
                        stop=(last and pt >= NPT - 4),
                        tile_position=(0, 32 * strip),
                        skip_group_check=True,
                    )

            out_sb = persist.tile([B, Q], fp32, tag="out_sb")
            nc.scalar.copy(out_sb[:], psum[0:B, :])
            for strip in range(1, 4):
                nc.vector.tensor_tensor(
                    out_sb[:], out_sb[:], psum[32 * strip : 32 * strip + B, :], Alu.add
                )
            nc.sync.dma_start(out=out_d[:], in_=out_sb[:])

    return nc


def _split_multi_waits(nc):
    """Walrus encodes at most one sync-wait per 64B instruction for several
    TRN2 instruction formats; Tile can attach two. Move excess waits onto
    injected same-engine NoOp carriers placed immediately before."""
    import concourse.mybir as mybir

    for fn in nc.m.functions:
        for bb in fn.blocks:
            il = bb.instructions
            out = []
            changed = False
            for ins in il:
                si = ins.sync_info
                if si is not None and si.on_wait and len(si.on_wait) > 1:
                    waits = list(si.on_wait)
                    for w in waits[:-1]:
                        out.append(
                            mybir.InstNoOp(
                                name=nc.get_next_instruction_name(),
                                engine=ins.engine,
                                ins=[],
                                outs=[],
                                sync_info=mybir.SyncInfo(on_wait=[w], on_update=[]),
                            )
                        )
                    ins.sync_info = mybir.SyncInfo(
                        on_wait=[waits[-1]], on_update=list(si.on_update or [])
                    )
                    changed = True
                out.append(ins)
            if changed:
                il[:] = out


def _get_program(split_waits=True):
    # split_waits=False is for CoreSim runs (its race detector can't digest
    # post-hoc injected NoOps); hardware compiles need the split.
    key = ("nc", split_waits)
    if key not in _CACHE:
        nc = _build_program()
        if split_waits:
            _split_multi_waits(nc)
        _CACHE[key] = nc
    return _CACHE[key]


def _stt_np_dtype():
    if STT_BF16:
        import ml_dtypes
        return ml_dtypes.bfloat16
    return np.float16


def _host_layouts(buf, weight, delay_raw):
    # bufT[pr, pt, t, b] = buf[b, t, pt*128+pr], flattened to [128, NPT*T*B]
    bufT = (
        np.ascontiguousarray(
            buf.transpose(2, 1, 0)  # [P, T, B]
            .reshape(NPT, 128, T, B)
            .transpose(1, 0, 2, 3)  # [128, NPT, T, B]
        )
        .reshape(128, NPT * T * B)
        .astype(np.float16)
    )
    # per-core column slices, [128, NPT, Q] -> [128, FD]
    ws, ds = [], []
    for c in range(NCORES):
        wq = weight[:, c * Q : (c + 1) * Q].reshape(NPT, 128, Q).transpose(1, 0, 2)
        dq = delay_raw[:, c * Q : (c + 1) * Q].reshape(NPT, 128, Q).transpose(1, 0, 2)
        ws.append(np.ascontiguousarray(wq).reshape(128, FD).astype(_stt_np_dtype()))
        ds.append(np.ascontiguousarray(dq).reshape(128, FD).astype(np.float32))
    return bufT, ws, ds


def kernel(buf, weight, delay_raw):
    from concourse.bass_utils import run_bass_kernel_spmd

    buf = np.asarray(buf, dtype=np.float32)
    weight = np.asarray(weight, dtype=np.float32)
    delay_raw = np.asarray(delay_raw, dtype=np.float32)

    nc = _get_program()
    bufT, ws, ds = _host_layouts(buf, weight, delay_raw)
    in_maps = [
        {"buft": bufT, "w": ws[c], "delay": ds[c]} for c in range(NCORES)
    ]
    last_err = None
    for _attempt in range(3):
        try:
            res = run_bass_kernel_spmd(nc, in_maps, core_ids=list(range(NCORES)))
            break
        except Exception as e:  # transient NRT_EXEC_UNIT_UNRECOVERABLE faults
            last_err = e
    else:
        raise last_err
    out = np.concatenate([res.results[c]["out"] for c in range(NCORES)], axis=1)
    return out.astype(np.float32)


if __name__ == "__main__":
    rng = np.random.default_rng(0)
    buf = rng.random((B, T, P), dtype=np.float32)
    weight = rng.standard_normal((P, QFULL), dtype=np.float32) * np.sqrt(2.0 / P)
    delay_raw = rng.standard_normal((P, QFULL), dtype=np.float32)
    out = kernel(buf=buf, weight=weight, delay_raw=delay_raw)
    print("out", out.shape, out.dtype, float(np.abs(out).max()))
